# revision 3
# baseline (speedup 1.0000x reference)
"""Trainium2 Bass kernel for nn_DiffusionNet (dense transformer, B=1024).

Strategy: pure data-parallel over batch (8 cores x 128 rows). Per core all
activations live in SBUF as batch-major [128, D] tiles (the n=4 sequence
positions are 4 separate tiles). Matmuls y = x @ W run as psum = lhsT.T @ rhs
with lhsT = x^T chunks [128k, 128b] (PE transposes, rounded to fp32r) and
rhs = W tiles [128, 512] streamed from DRAM in fp32r (full rate on TRN2 for
free-dim >= 256). Weight tiles are shared across the 4 token matmuls. LN
gains are folded into the following weight matrices on the host; LN stats
use bn_stats/bn_aggr. Attention (4 tokens, j<=5 incl. the learned null kv,
single-head kv) runs on the DVE with broadcast APs and segmented reduces.
"""
import math

import numpy as np

try:
    import concourse.bass as bass
except ImportError:  # grading env fallback
    import sys
    sys.path.insert(0, "/opt/trn_rl_repo")
    import concourse.bass as bass

import concourse.bacc as bacc
import concourse.mybir as mybir
import concourse.tile as tile
from concourse import bass_utils  # noqa: F401  (kept for debugging)

FP32 = mybir.dt.float32
FP32R = mybir.dt.float32r
AF = mybir.ActivationFunctionType
OP = mybir.AluOpType
AX = mybir.AxisListType

B, DIM, ZDIM, TDIM = 1024, 1024, 512, 768
RATIO, RESDEPTH, DEPTH = 4, 3, 4
HEADS, DH = 16, 64
INNER = HEADS * DH
FF_INNER = 4 * DIM
NBUCKETS, MAXDIST = 32, 128
ROT_DIM = 32
EPS = 1e-5
NCORES = 8
BS = B // NCORES
P = 128
NL = DEPTH + 2
NTOK = 4

_CACHE = {}


# ----------------------------------------------------------------------------
# host-side prep
# ----------------------------------------------------------------------------

def _np(a):
    return np.ascontiguousarray(np.asarray(a, dtype=np.float32))


def _rel_pos_bias_host(emb):
    i, j = NTOK, NTOK + 1
    q_pos = np.arange(j - i, j)
    k_pos = np.arange(j)
    rel = k_pos[None, :] - q_pos[:, None]
    n = np.maximum(-rel, 0)
    max_exact = NBUCKETS // 2
    val_large = max_exact + (
        np.log(np.maximum(n, 1).astype(np.float32) / max_exact)
        / np.log(MAXDIST / max_exact) * (NBUCKETS - max_exact)
    ).astype(np.int32)
    val_large = np.minimum(val_large, NBUCKETS - 1)
    bucket = np.where(n < max_exact, n, val_large)
    return _np(emb[bucket])  # [i, j, h]


def _sin_emb_host(t):
    half = DIM // 2
    emb = np.exp(np.arange(half, dtype=np.float32) * (-math.log(10000.0) / (half - 1)))
    args = np.asarray(t, np.float32)[:, None] * emb[None, :]
    return _np(np.concatenate([np.sin(args), np.cos(args)], axis=-1))


def _prep_arrays(params):
    g = {}

    def res_block(prefix, p):
        g[prefix + "_w0"] = _np(p["w0"])
        g[prefix + "_b0"] = _np(p["b0"]).reshape(1, -1)
        g[prefix + "_w1"] = _np(p["w1"])
        g[prefix + "_b1"] = _np(p["b1"]).reshape(1, -1)
        if "ws" in p:
            g[prefix + "_ws"] = _np(p["ws"])

    for i, p in enumerate(params["z_resnet"]):
        res_block(f"zb{i}", p)
    for i, p in enumerate(params["text_resnet"]):
        res_block(f"tb{i}", p)
    m = params["time_mlp"]
    g["tm_w1"] = _np(m["w1"])
    g["tm_b1"] = _np(m["b1"]).reshape(1, -1)
    g["tm_w2"] = _np(m["w2"])
    g["tm_b2"] = _np(m["b2"]).reshape(1, -1)
    g["lq"] = _np(params["learned_query"]).reshape(1, -1)
    g["relbias"] = _rel_pos_bias_host(_np(params["rel_pos_emb"])).reshape(
        NTOK, 5 * HEADS)

    inv = 1.0 / (10000.0 ** (np.arange(0, ROT_DIM, 2, dtype=np.float32) / ROT_DIM))
    f = np.arange(NTOK, dtype=np.float32)[:, None] * inv[None, :]
    g["rotcos"] = _np(np.cos(f))
    g["rotsin"] = _np(np.sin(f))

    scale = DH ** -0.5
    for l, lp in enumerate(params["layers"]):
        for kind in ("self", "cross"):
            p = lp[kind]
            kk = "s" if kind == "self" else "c"
            ng = _np(p["norm_g"])
            g[f"l{l}{kk}_wq"] = _np(ng[:, None] * _np(p["wq"]) * scale)
            if kind == "self":
                g[f"l{l}{kk}_wkv"] = _np(ng[:, None] * _np(p["wkv"]))
            else:
                g[f"l{l}{kk}_wkv"] = _np(p["wkv"])
            g[f"l{l}{kk}_nkv"] = _np(p["null_kv"])
            g[f"l{l}{kk}_wo"] = _np(p["wo"])
            g[f"l{l}{kk}_og"] = _np(p["out_norm_g"]).reshape(1, -1)
        p = lp["ff"]
        ng = _np(p["norm_g"])
        g[f"l{l}f_w1"] = _np(ng[:, None] * _np(p["w1"]))
        g[f"l{l}f_w2"] = _np(p["w2"])

    g["proj"] = _np(_np(params["out_norm_g"])[:, None] * _np(params["proj_out"]))
    g["ident"] = np.eye(P, dtype=np.float32)
    return g


def _is_weight(name):
    return (
        name.endswith(("_w0", "_w1", "_w2", "_ws", "_wq", "_wkv", "_wo"))
        or name == "proj"
    )


def _bcast_dram(ap, n, w):
    return bass.AP(tensor=ap.tensor, offset=ap.offset, ap=[[0, n], [1, w]])


# ----------------------------------------------------------------------------
# device program
# ----------------------------------------------------------------------------

def _build(shared_shapes):
    nc = bacc.Bacc(trn_type="TRN2", num_devices=NCORES, debug=False)
    D = {}
    for name, shp in shared_shapes.items():
        dt = FP32R if _is_weight(name) else FP32
        D[name] = nc.dram_tensor(name, list(shp), dt, kind="ExternalInput").ap()
    D["data"] = nc.dram_tensor("data", [BS, DIM], FP32, kind="ExternalInput").ap()
    D["z_hat"] = nc.dram_tensor("z_hat", [BS, ZDIM], FP32, kind="ExternalInput").ap()
    D["text"] = nc.dram_tensor("text", [BS, TDIM], FP32, kind="ExternalInput").ap()
    D["semb"] = nc.dram_tensor("semb", [BS, DIM], FP32, kind="ExternalInput").ap()
    out_d = nc.dram_tensor("out", [BS, DIM], FP32, kind="ExternalOutput").ap()

    with tile.TileContext(nc) as tc:
        _program(nc, tc, D, out_d)
    nc.compile()
    return nc


def _program(nc, tc, D, out_d):
    import contextlib
    ctx = contextlib.ExitStack()
    with ctx:
        persist = ctx.enter_context(tc.tile_pool(name="persist", bufs=1))
        small = ctx.enter_context(tc.tile_pool(name="small", bufs=4))
        wp = ctx.enter_context(tc.tile_pool(name="wp", bufs=4))
        pp = ctx.enter_context(tc.tile_pool(name="pp", bufs=1, space="PSUM"))
        tpp = ctx.enter_context(tc.tile_pool(name="tpp", bufs=4, space="PSUM"))

        ident = persist.tile([P, P], FP32)
        nc.sync.dma_start(out=ident, in_=D["ident"])
        eps_t = persist.tile([P, 1], FP32)
        nc.vector.memset(eps_t, EPS)

        # ----- helpers -----
        def transpose_to(dst, src, kc):
            """dst: fp32r [P, kc, P]; src: AP [P, kc*P] batch-major."""
            for k in range(kc):
                tp = tpp.tile([P, P], FP32, tag="tp", name="tp", bufs=4)
                nc.tensor.transpose(tp, src[:, k * P:(k + 1) * P], ident)
                nc.scalar.copy(out=dst[:, k, :], in_=tp)

        def new_T(pool, src, kc, tag):
            t = pool.tile([P, kc, P], FP32R, tag=tag, name=tag, bufs=1)
            transpose_to(t, src, kc)
            return t

        def mm(srcs, dout, epilogue, ntok=1, col_map=None):
            """srcs: list of (w_dram_ap, [lhsT APs per token], kc)."""
            ng = (dout + 511) // 512
            for gi in range(ng):
                w = min(512, dout - gi * 512)
                c0 = col_map(gi) if col_map else gi * 512
                accs = [
                    pp.tile([P, w], FP32, tag=f"acc{t}", name=f"acc{t}", bufs=1)
                    for t in range(ntok)
                ]
                nsrc = len(srcs)
                for si, (wd, lhsTs, kc) in enumerate(srcs):
                    for k in range(kc):
                        wt = wp.tile([P, w], FP32R, tag="wt", name="wt", bufs=4)
                        nc.sync.dma_start(
                            out=wt, in_=wd[k * P:(k + 1) * P, c0:c0 + w])
                        st = si == 0 and k == 0
                        sp = si == nsrc - 1 and k == kc - 1
                        for t in range(ntok):
                            nc.tensor.matmul(
                                accs[t], lhsTs[t][:, k, :], wt, start=st, stop=sp)
                epilogue(gi, w, accs)

        def ln_stats(x, dwidth):
            nsub = dwidth // 512
            st = small.tile([P, nsub, 6], FP32, tag="bnst", name="st")
            for s in range(nsub):
                nc.vector.bn_stats(out=st[:, s, :], in_=x[:, s * 512:(s + 1) * 512])
            mv = small.tile([P, 2], FP32, tag="mv", name="mv")
            nc.vector.bn_aggr(out=mv, in_=st)
            rstd = small.tile([P, 1], FP32, tag="rstd", name="rstd")
            nc.scalar.activation(rstd, mv[:, 1:2], AF.Abs_reciprocal_sqrt,
                                 bias=eps_t, scale=1.0)
            nb = small.tile([P, 1], FP32, tag="nb", name="nb")
            nc.vector.scalar_tensor_tensor(nb, mv[:, 0:1], -1.0, rstd,
                                           OP.mult, OP.mult)
            return rstd, nb

        def ln_apply(out, x, dwidth):
            rstd, nb = ln_stats(x, dwidth)
            nc.scalar.activation(out, x, AF.Identity, bias=nb, scale=rstd)

        # ----- conditioning branches (scoped pools) -----
        x0 = persist.tile([P, DIM], FP32, name="x0")
        x1 = persist.tile([P, DIM], FP32, name="x1")
        x2 = persist.tile([P, DIM], FP32, name="x2")
        x3 = persist.tile([P, DIM], FP32, name="x3")
        nc.sync.dma_start(out=x2, in_=D["data"])
        nc.sync.dma_start(out=x3, in_=_bcast_dram(D["lq"], P, DIM))
        xt = [x0, x1, x2, x3]
        t_sb = persist.tile([P, DIM * RATIO], FP32, name="t_sb")

        def resnet_block(sp, pref, x_sb, din, dout, out_sb, has_ws):
            h = min(din, dout)
            r = sp.tile([P, din], FP32, tag="res_r", name="r", bufs=1)
            nc.scalar.activation(r, x_sb, AF.Relu)
            rT = new_T(sp, r, din // P, "rT")
            b0 = sp.tile([P, h], FP32, tag="res_b0", name="b0", bufs=1)
            nc.sync.dma_start(out=b0, in_=_bcast_dram(D[pref + "_b0"], P, h))
            r2 = sp.tile([P, h], FP32, tag="res_r2", name="r2", bufs=1)

            def ep0(gi, w, accs):
                sl = slice(gi * 512, gi * 512 + w)
                nc.vector.tensor_tensor(out=r2[:, sl], in0=accs[0],
                                        in1=b0[:, sl], op=OP.add)
                nc.vector.tensor_relu(out=r2[:, sl], in_=r2[:, sl])

            mm([(D[pref + "_w0"], [rT], din // P)], h, ep0)
            r2T = new_T(sp, r2, h // P, "r2T")
            srcs = [(D[pref + "_w1"], [r2T], h // P)]
            if has_ws:
                xT = new_T(sp, x_sb, din // P, "xsT")
                srcs.append((D[pref + "_ws"], [xT], din // P))
            b1 = sp.tile([P, dout], FP32, tag="res_b1", name="b1", bufs=1)
            nc.sync.dma_start(out=b1, in_=_bcast_dram(D[pref + "_b1"], P, dout))

            def ep1(gi, w, accs):
                sl = slice(gi * 512, gi * 512 + w)
                nc.vector.tensor_tensor(out=out_sb[:, sl], in0=accs[0],
                                        in1=b1[:, sl], op=OP.add)
                if not has_ws:
                    nc.vector.tensor_tensor(out=out_sb[:, sl], in0=out_sb[:, sl],
                                            in1=x_sb[:, sl], op=OP.add)

            mm(srcs, dout, ep1)

        with tc.tile_pool(name="zscope", bufs=1) as zp:
            z_in = zp.tile([P, ZDIM], FP32, name="z_in")
            nc.sync.dma_start(out=z_in, in_=D["z_hat"])
            zb = zp.tile([P, DIM], FP32, name="zb")
            zb2 = zp.tile([P, DIM], FP32, name="zb2")
            with tc.tile_pool(name="zs0", bufs=1) as sp:
                resnet_block(sp, "zb0", z_in, ZDIM, DIM, zb, True)
            with tc.tile_pool(name="zs1", bufs=1) as sp:
                resnet_block(sp, "zb1", zb, DIM, DIM, zb2, False)
            with tc.tile_pool(name="zs2", bufs=1) as sp:
                resnet_block(sp, "zb2", zb2, DIM, DIM, x0, False)

        with tc.tile_pool(name="tmscope", bufs=1) as tp_:
            se = tp_.tile([P, DIM], FP32, name="se")
            nc.sync.dma_start(out=se, in_=D["semb"])
            seT = new_T(tp_, se, DIM // P, "seT")
            b1b = tp_.tile([P, DIM], FP32, name="tmb1")
            nc.sync.dma_start(out=b1b, in_=_bcast_dram(D["tm_b1"], P, DIM))
            te1 = tp_.tile([P, DIM], FP32, name="te1")

            def ep_tm1(gi, w, accs):
                sl = slice(gi * 512, gi * 512 + w)
                nc.vector.tensor_tensor(out=te1[:, sl], in0=accs[0],
                                        in1=b1b[:, sl], op=OP.add)
                nc.scalar.activation(te1[:, sl], te1[:, sl], AF.Silu)

            mm([(D["tm_w1"], [seT], DIM // P)], DIM, ep_tm1)
            te1T = new_T(tp_, te1, DIM // P, "te1T")
            b2b = tp_.tile([P, DIM], FP32, name="tmb2")
            nc.sync.dma_start(out=b2b, in_=_bcast_dram(D["tm_b2"], P, DIM))

            def ep_tm2(gi, w, accs):
                sl = slice(gi * 512, gi * 512 + w)
                nc.vector.tensor_tensor(out=x1[:, sl], in0=accs[0],
                                        in1=b2b[:, sl], op=OP.add)

            mm([(D["tm_w2"], [te1T], DIM // P)], DIM, ep_tm2)

        with tc.tile_pool(name="tscope", bufs=1) as txp:
            t_in = txp.tile([P, TDIM], FP32, name="t_in")
            nc.sync.dma_start(out=t_in, in_=D["text"])
            t_sb2 = txp.tile([P, DIM * RATIO], FP32, name="t_sb2")
            with tc.tile_pool(name="ts0", bufs=1) as sp:
                resnet_block(sp, "tb0", t_in, TDIM, DIM * RATIO, t_sb, True)
            with tc.tile_pool(name="ts1", bufs=1) as sp:
                resnet_block(sp, "tb1", t_sb, DIM * RATIO, DIM * RATIO, t_sb2, False)
            with tc.tile_pool(name="ts2", bufs=1) as sp:
                resnet_block(sp, "tb2", t_sb2, DIM * RATIO, DIM * RATIO, t_sb, False)

        ctxT = persist.tile([P, NTOK, DIM // P, P], FP32R, name="ctxT")
        for t in range(NTOK):
            transpose_to(ctxT[:, t], t_sb[:, t * DIM:(t + 1) * DIM], DIM // P)

        # ----- attention tables -----
        relb = persist.tile([P, NTOK, 5 * HEADS], FP32, name="relb")
        rcos = persist.tile([P, NTOK, 16], FP32, name="rcos")
        rsin = persist.tile([P, NTOK, 16], FP32, name="rsin")
        for i in range(NTOK):
            nc.sync.dma_start(out=relb[:, i, :],
                              in_=_bcast_dram(D["relbias"][i:i + 1, :], P, 5 * HEADS))
            nc.sync.dma_start(out=rcos[:, i, :],
                              in_=_bcast_dram(D["rotcos"][i:i + 1, :], P, 16))
            nc.sync.dma_start(out=rsin[:, i, :],
                              in_=_bcast_dram(D["rotsin"][i:i + 1, :], P, 16))

        def rotary_inplace(q, t, nheads):
            qv = q.rearrange("p (h d) -> p h d", d=DH)
            x1v = qv[:, :, 0:16]
            x2v = qv[:, :, 16:32]
            cosb = rcos[:, t, :].unsqueeze(1).broadcast_to([P, nheads, 16])
            sinb = rsin[:, t, :].unsqueeze(1).broadcast_to([P, nheads, 16])
            u2 = small.tile([P, nheads, 16], FP32, tag=f"u2_{nheads}", name="u2")
            u1 = small.tile([P, nheads, 16], FP32, tag=f"u1_{nheads}", name="u1")
            nc.vector.tensor_tensor(out=u2, in0=x2v, in1=sinb, op=OP.mult)
            nc.vector.tensor_tensor(out=u1, in0=x1v, in1=sinb, op=OP.mult)
            nc.vector.tensor_tensor(out=x1v, in0=x1v, in1=cosb, op=OP.mult)
            nc.vector.tensor_tensor(out=x2v, in0=x2v, in1=cosb, op=OP.mult)
            nc.vector.tensor_tensor(out=x1v, in0=x1v, in1=u2, op=OP.subtract)
            nc.vector.tensor_tensor(out=x2v, in0=x2v, in1=u1, op=OP.add)

        # ----- transformer -----
        def attention(ap_, l, kind):
            kk = "s" if kind == "self" else "c"
            pref = f"l{l}{kk}"
            xnT = []
            for t in range(NTOK):
                xn = ap_.tile([P, DIM], FP32, tag="xn", name="xn", bufs=2)
                ln_apply(xn, xt[t], DIM)
                xnT.append(new_T(ap_, xn, DIM // P, f"xnT{t}"))

            q = [ap_.tile([P, INNER], FP32, tag=f"q{t}", name=f"q{t}", bufs=1)
                 for t in range(NTOK)]

            def ep_q(gi, w, accs):
                for t in range(NTOK):
                    nc.scalar.copy(out=q[t][:, gi * 512:gi * 512 + w], in_=accs[t])

            mm([(D[pref + "_wq"], xnT, DIM // P)], INNER, ep_q, ntok=NTOK)
            for t in range(NTOK):
                rotary_inplace(q[t], t, HEADS)

            kvsrc = xnT if kind == "self" else [ctxT[:, t] for t in range(NTOK)]
            ks = [ap_.tile([P, DH], FP32, tag=f"k{t}", name=f"k{t}", bufs=1)
                  for t in range(NTOK)]
            vs = [ap_.tile([P, DH], FP32, tag=f"v{t}", name=f"v{t}", bufs=1)
                  for t in range(NTOK)]

            def ep_kv(gi, w, accs):
                for t in range(NTOK):
                    nc.scalar.copy(out=ks[t], in_=accs[t][:, 0:DH])
                    nc.scalar.copy(out=vs[t], in_=accs[t][:, DH:2 * DH])

            mm([(D[pref + "_wkv"], kvsrc, DIM // P)], 2 * DH, ep_kv, ntok=NTOK)
            for t in range(NTOK):
                rotary_inplace(ks[t], t, 1)

            nullk = ap_.tile([P, DH], FP32, tag="nullk", name="nullk", bufs=1)
            nullv = ap_.tile([P, DH], FP32, tag="nullv", name="nullv", bufs=1)
            nc.sync.dma_start(out=nullk,
                              in_=_bcast_dram(D[pref + "_nkv"][0:1, :], P, DH))
            nc.sync.dma_start(out=nullv,
                              in_=_bcast_dram(D[pref + "_nkv"][1:2, :], P, DH))
            kl = [nullk] + ks
            vl = [nullv] + vs

            oT = []
            for i in range(NTOK):
                nj = i + 2
                sc = ap_.tile([P, 5, HEADS], FP32, tag="sc", name="sc", bufs=2)
                prod = ap_.tile([P, INNER], FP32, tag="prod", name="prod", bufs=2)
                qv = q[i].rearrange("p (h d) -> p h d", d=DH)
                pv = prod.rearrange("p (h d) -> p h d", d=DH)
                for j in range(nj):
                    kb = kl[j].unsqueeze(1).broadcast_to([P, HEADS, DH])
                    nc.vector.tensor_tensor(out=pv, in0=qv, in1=kb, op=OP.mult)
                    nc.vector.tensor_reduce(out=sc[:, j, :], in_=pv,
                                            axis=AX.X, op=OP.add)
                scv = sc[:, 0:nj, :]
                if kind == "self":
                    nc.vector.tensor_tensor(
                        out=scv, in0=scv,
                        in1=relb[:, i, 0:nj * HEADS].rearrange(
                            "p (j h) -> p j h", h=HEADS),
                        op=OP.add)
                mx = small.tile([P, HEADS], FP32, tag="mx", name="mx")
                nc.vector.tensor_reduce(out=mx, in_=scv.transpose([0, 2, 1]),
                                        axis=AX.X, op=OP.max)
                nc.vector.tensor_tensor(
                    out=scv, in0=scv,
                    in1=mx.unsqueeze(1).broadcast_to([P, nj, HEADS]),
                    op=OP.subtract)
                nc.scalar.activation(scv, scv, AF.Exp)
                sm = small.tile([P, HEADS], FP32, tag="sm", name="sm")
                nc.vector.tensor_reduce(out=sm, in_=scv.transpose([0, 2, 1]),
                                        axis=AX.X, op=OP.add)
                nc.vector.reciprocal(out=sm, in_=sm)
                nc.vector.tensor_tensor(
                    out=scv, in0=scv,
                    in1=sm.unsqueeze(1).broadcast_to([P, nj, HEADS]),
                    op=OP.mult)
                o = ap_.tile([P, INNER], FP32, tag="o", name="o", bufs=2)
                ov = o.rearrange("p (h d) -> p h d", d=DH)
                for j in range(nj):
                    pb = sc[:, j, :].unsqueeze(2).broadcast_to([P, HEADS, DH])
                    vb = vl[j].unsqueeze(1).broadcast_to([P, HEADS, DH])
                    if j == 0:
                        nc.vector.tensor_tensor(out=ov, in0=pb, in1=vb, op=OP.mult)
                    else:
                        nc.vector.tensor_tensor(out=pv, in0=pb, in1=vb, op=OP.mult)
                        nc.vector.tensor_tensor(out=o, in0=o, in1=prod, op=OP.add)
                oT.append(new_T(ap_, o, INNER // P, f"oT{i}"))

            og = ap_.tile([P, DIM], FP32, tag="og", name="og", bufs=1)
            nc.sync.dma_start(out=og, in_=_bcast_dram(D[pref + "_og"], P, DIM))
            o2 = [ap_.tile([P, DIM], FP32, tag=f"o2_{t}", name=f"o2_{t}", bufs=1)
                  for t in range(NTOK)]

            def ep_wo(gi, w, accs):
                for t in range(NTOK):
                    nc.scalar.copy(out=o2[t][:, gi * 512:gi * 512 + w], in_=accs[t])

            mm([(D[pref + "_wo"], oT, DIM // P)], DIM, ep_wo, ntok=NTOK)
            for t in range(NTOK):
                rstd, nb = ln_stats(o2[t], DIM)
                u = ap_.tile([P, DIM], FP32, tag="u_ln", name="u", bufs=2)
                nc.scalar.activation(u, o2[t], AF.Identity, bias=nb, scale=rstd)
                nc.vector.tensor_tensor(out=u, in0=u, in1=og, op=OP.mult)
                nc.vector.tensor_tensor(out=xt[t], in0=xt[t], in1=u, op=OP.add)

        def feedforward(fp_, l):
            pref = f"l{l}f"
            xnT = []
            for t in range(NTOK):
                xn = fp_.tile([P, DIM], FP32, tag="xn", name="xn", bufs=2)
                ln_apply(xn, xt[t], DIM)
                xnT.append(new_T(fp_, xn, DIM // P, f"xnT{t}"))
            hT = [fp_.tile([P, FF_INNER // P, P], FP32R, tag=f"hT{t}",
                           name=f"hT{t}", bufs=1) for t in range(NTOK)]
            ha = [fp_.tile([P, 512], FP32, tag=f"ha{t}", name=f"ha{t}", bufs=1)
                  for t in range(NTOK)]

            def col_map(gi):
                half, idx = gi % 2, gi // 2
                return half * FF_INNER + idx * 512

            def ep_w1(gi, w, accs):
                half, idx = gi % 2, gi // 2
                if half == 0:
                    for t in range(NTOK):
                        nc.scalar.copy(out=ha[t], in_=accs[t])
                else:
                    for t in range(NTOK):
                        sg = fp_.tile([P, 512], FP32, tag="sg", name="sg", bufs=2)
                        nc.scalar.activation(sg, accs[t], AF.Silu)
                        hh = fp_.tile([P, 512], FP32, tag="hh", name="hh", bufs=2)
                        nc.vector.tensor_tensor(out=hh, in0=ha[t], in1=sg,
                                                op=OP.mult)
                        transpose_to(hT[t][:, idx * 4:(idx + 1) * 4], hh, 4)

            mm([(D[pref + "_w1"], xnT, DIM // P)], 2 * FF_INNER, ep_w1,
               ntok=NTOK, col_map=col_map)

            def ep_w2(gi, w, accs):
                sl = slice(gi * 512, gi * 512 + w)
                for t in range(NTOK):
                    nc.vector.tensor_tensor(out=xt[t][:, sl], in0=xt[t][:, sl],
                                            in1=accs[t], op=OP.add)

            mm([(D[pref + "_w2"], hT, FF_INNER // P)], DIM, ep_w2, ntok=NTOK)

        for l in range(NL):
            with tc.tile_pool(name=f"attn_s{l}", bufs=1) as ap_:
                attention(ap_, l, "self")
            with tc.tile_pool(name=f"attn_c{l}", bufs=1) as ap_:
                attention(ap_, l, "cross")
            with tc.tile_pool(name=f"ff{l}", bufs=1) as fp_:
                feedforward(fp_, l)

        # ----- final stable LN + proj (token 3 only) -----
        with tc.tile_pool(name="fin", bufs=1) as fin:
            mxf = small.tile([P, 1], FP32, tag="mxf", name="mxf")
            nc.vector.tensor_reduce(out=mxf, in_=x3, axis=AX.X, op=OP.max)
            nc.vector.reciprocal(out=mxf, in_=mxf)
            xs = fin.tile([P, DIM], FP32, name="xs")
            nc.scalar.activation(xs, x3, AF.Identity, scale=mxf)
            xn = fin.tile([P, DIM], FP32, name="xnf")
            ln_apply(xn, xs, DIM)
            xnT = new_T(fin, xn, DIM // P, "xnTf")
            out_sb = fin.tile([P, DIM], FP32, name="out_sb")

            def ep_proj(gi, w, accs):
                nc.scalar.copy(out=out_sb[:, gi * 512:gi * 512 + w], in_=accs[0])

            mm([(D["proj"], [xnT], DIM // P)], DIM, ep_proj)
            nc.sync.dma_start(out=out_d, in_=out_sb)


# ----------------------------------------------------------------------------
# runner (jit once, reuse across calls)
# ----------------------------------------------------------------------------

def _get_runner(shared_shapes):
    key = tuple(sorted((k, tuple(v)) for k, v in shared_shapes.items()))
    if _CACHE.get("key") == key:
        return _CACHE["runner"]

    nc = _build(shared_shapes)

    import jax
    from jax.sharding import Mesh, PartitionSpec, NamedSharding
    from jax.experimental.shard_map import shard_map
    from concourse import bass2jax

    bass2jax.install_neuronx_cc_hook()

    partition_name = nc.partition_id_tensor.name if nc.partition_id_tensor else None
    in_names, out_names, out_avals, zero_outs = [], [], [], []
    for alloc in nc.m.functions[0].allocations:
        if not isinstance(alloc, mybir.MemoryLocationSet):
            continue
        name = alloc.memorylocations[0].name
        if alloc.kind == "ExternalInput":
            if name != partition_name:
                in_names.append(name)
        elif alloc.kind == "ExternalOutput":
            shape = tuple(alloc.tensor_shape)
            dtype = mybir.dt.np(alloc.dtype)
            out_names.append(name)
            out_avals.append(jax.core.ShapedArray(shape, dtype))
            zero_outs.append(np.zeros(shape, dtype))
    n_params = len(in_names)
    all_in_names = list(in_names) + list(out_names)
    if partition_name is not None:
        all_in_names.append(partition_name)
    donate = tuple(range(n_params, n_params + len(out_names)))

    def _body(*args):
        operands = list(args)
        if partition_name is not None:
            operands.append(bass2jax.partition_id_tensor())
        outs = bass2jax._bass_exec_p.bind(
            *operands,
            out_avals=tuple(out_avals),
            in_names=tuple(all_in_names),
            out_names=tuple(out_names),
            lowering_input_output_aliases=(),
            sim_require_finite=True,
            sim_require_nnan=True,
            nc=nc,
        )
        return tuple(outs)

    devices = jax.devices()[:NCORES]
    mesh = Mesh(np.asarray(devices), ("core",))
    in_specs = (PartitionSpec("core"),) * (n_params + len(out_names))
    out_specs = (PartitionSpec("core"),) * len(out_names)
    jfn = jax.jit(
        shard_map(_body, mesh=mesh, in_specs=in_specs, out_specs=out_specs,
                  check_rep=False),
        donate_argnums=donate, keep_unused=True)
    sharding = NamedSharding(mesh, PartitionSpec("core"))

    def put(per_core_arrays):
        singles = [
            jax.device_put(per_core_arrays[c], devices[c]) for c in range(NCORES)
        ]
        shp = per_core_arrays[0].shape
        gshape = (NCORES * shp[0],) + tuple(shp[1:])
        return jax.make_array_from_single_device_arrays(gshape, sharding, singles)

    def stage(in_map_shared, in_map_per_core):
        args = []
        for name in in_names:
            if name in in_map_per_core:
                args.append(put(in_map_per_core[name]))
            else:
                args.append(put([in_map_shared[name]] * NCORES))
        return args

    def execute(args):
        zouts = [put([z] * NCORES) for z in zero_outs]
        outs = jfn(*args, *zouts)
        return {name: np.asarray(outs[i]) for i, name in enumerate(out_names)}

    runner = {"stage": stage, "execute": execute, "jfn": jfn, "put": put,
              "in_names": in_names, "out_names": out_names,
              "zero_outs": zero_outs, "nc": nc}
    _CACHE["key"] = key
    _CACHE["runner"] = runner
    return runner


def kernel(data, z_hat, text, diffusion_timesteps, params):
    data = _np(data)
    z_hat = _np(z_hat)
    text = _np(text)
    ts = _np(diffusion_timesteps)
    shared = _prep_arrays(params)
    semb = _sin_emb_host(ts)

    shapes = {k: v.shape for k, v in shared.items()}
    runner = _get_runner(shapes)

    per_core = {
        "data": [data[c * BS:(c + 1) * BS] for c in range(NCORES)],
        "z_hat": [z_hat[c * BS:(c + 1) * BS] for c in range(NCORES)],
        "text": [text[c * BS:(c + 1) * BS] for c in range(NCORES)],
        "semb": [semb[c * BS:(c + 1) * BS] for c in range(NCORES)],
    }
    args = runner["stage"](shared, per_core)
    res = runner["execute"](args)
    return res["out"]
